# revision 17
# baseline (speedup 1.0000x reference)
"""GATv2 2-layer + down-proj kernel for Trainium2, 8 NeuronCores.

Strategy (edge/data parallel, dst-sorted, v2):
- Add self loops; nodes padded to 50176 = 8*6272; core c owns dst nodes
  [c*6272, (c+1)*6272) = 49 blocks of 128 nodes. Edges sorted by dst block;
  within a block sorted by (layer-specific) source table row.
- Layer 0: every core computes the FULL xl0 table locally from the full
  x.T input (no collective), plus the xr0 slab for its own nodes
  (kept resident in SBUF).
- Layer 1: per-block fused projections (xl1/xr1 from h right after each
  block's aggregation); xl1 slab chunks AllGathered (7 chunks of 7 blocks)
  into a chunk-major full table while later blocks still compute, hiding
  most of the collective behind layer-0 edge work.
- Edge phase per 128-dst-node block (21 = 13 lo + 8 hi edge tiles of 128):
    XL = dma_gather(xl_table[src])                      (SWDGE, bf16 rows)
    ET_ps[f,slot] = xr_blk.T-matmul(ST) + XL.T          (PE, one psum chunk
      per 4 tiles: RT = lhsT(xr_blk) x rhs(ST fp8), then XL passthrough-
      transpose via identity rhs, accumulated)
    ET = prelu(ET_ps)                                   (ACT)
    lg[slot,h] = matmul(lhsT=ET_tile, rhs=att)          (PE, N=4)
    M[:,128:132] = exp(lg)                              (ACT, one instr;
      softmax max-subtraction omitted: logits are O(1) by construction)
    M[:,0:128] = XL * ex (head-broadcast)               (DVE)
    acc = sum_t matmul(lhsT=S fp8, rhs=M)               (PE, psum accum)
    h = acc[:,0:128] * recip(acc[:,128:132])            (DVE)
- S (slot->dst one-hot) and ST (its transpose) are host-built fp8,
  packed together with the wrapped gather indices into one per-block
  "edata" row so each block does a single staging DMA.
- Biases folded on host: b0 -> c1 = b0 @ (Wl1+Wr1).T added to xr1 slab;
  b1, down_b -> cd = b1 @ down_W.T + down_b added at the output.
"""

import os
import sys

sys.path.insert(0, "/opt/trn_rl_repo")

import numpy as np
import ml_dtypes

import concourse.bass as bass
from concourse import bacc
import concourse.mybir as mybir
import concourse.tile as tile
from concourse.tile import add_dep_helper as _adh


def add_dep(a, b, reason=""):
    ia = a.ins if hasattr(a, "ins") else a
    ib = b.ins if hasattr(b, "ins") else b
    _adh(ia, ib, reason=reason)

from concourse.bass_utils import run_bass_kernel_spmd

F32 = mybir.dt.float32
BF16 = mybir.dt.bfloat16
I16 = mybir.dt.int16
FP8 = mybir.dt.float8e4
U8 = mybir.dt.uint8
AF = mybir.ActivationFunctionType
BF = ml_dtypes.bfloat16

N, E, DIN, H, C = 50000, 800000, 256, 4, 32
HID = H * C  # 128
NEG = 0.2
NCORES = 8
NBLK = 49                  # node blocks per core
NPC = NBLK * 128           # 6272 nodes per core
NPAD = NCORES * NPC        # 50176
SPLIT = 32768              # int16 gather table split
LO_T, HI_T = 13, 8
TPB = LO_T + HI_T          # 21 edge tiles / block
GBLK = NCORES * NBLK       # 392 global blocks
CHBS = [10, 10, 10, 10, 9]   # layer-1 allgather chunk sizes (blocks)
KCH = len(CHBS)
CHB_BASE = np.concatenate([[0], np.cumsum(CHBS)])          # block boundaries
CHROW_BASE = CHB_BASE * 128 * NCORES                       # table row base/chunk
# lo table rows [0, SPLIT) live in chunks 0..3; hi rows in chunks 3..4
LO_AGS = [c for c in range(KCH) if CHROW_BASE[c] < SPLIT]
HI_AGS = [c for c in range(KCH) if CHROW_BASE[c + 1] > SPLIT]
IXB = (LO_T + HI_T) * 8 * 2      # idx bytes per edata row (336)
EB = IXB + 2 * TPB * 128         # edata bytes per row (idx + S + ST)
GCH = 8                    # max tiles (x128 idx) per gather instruction

_CACHE = {}


def _wrap_idx(ix):
    """int [G, n] -> [G, 128, n//16]: idx i at [i%16, i//16], tiled 8x."""
    G, n = ix.shape
    out = np.zeros((G, 16, n // 16), np.int16)
    out[:, np.arange(n) % 16, np.arange(n) // 16] = ix.astype(np.int16)
    return np.tile(out, (1, 8, 1))


def _row0(node):
    """layer-0 table row: (p, j)-swapped within each 1024-row group so the
    phase-A table writes have 2048B contiguous runs per partition."""
    g, rem = node // 1024, node % 1024
    j, p = rem // 128, rem % 128
    return g * 1024 + p * 8 + j


def _row1(node):
    """layer-1 chunk-major table row for a node."""
    cc, rem = node // NPC, node % NPC
    lb, p = rem // 128, rem % 128
    c = np.searchsorted(CHB_BASE, lb, side="right") - 1
    return (CHROW_BASE[c] + cc * (np.asarray(CHBS)[c] * 128)
            + (lb - CHB_BASE[c]) * 128 + p)


def _build_layer_edata(src, dst, rows):
    """Section + one-hot build for one layer's source-row numbering.
    Returns edata [GBLK, 128, EB] uint8 (wrapped idx | S fp8 | ST fp8)."""
    blk = dst // 128
    order = np.lexsort((rows, blk))
    rs, ds, bs = rows[order], dst[order], blk[order]
    bounds = np.searchsorted(bs, np.arange(GBLK + 1))
    ix_lo = np.zeros((GBLK, LO_T * 128), np.int64)
    ix_hi = np.zeros((GBLK, HI_T * 128), np.int64)
    s_g, s_slot, s_col = [], [], []
    for g in range(GBLK):
        a, b = bounds[g], bounds[g + 1]
        r = rs[a:b]
        d = ds[a:b] - g * 128
        n_lo = int(np.searchsorted(r, SPLIT))
        n_hi = (b - a) - n_lo
        if n_lo > LO_T * 128 or n_hi > HI_T * 128:
            raise RuntimeError(f"block {g} sections overflow: {n_lo} {n_hi}")
        ix_lo[g, :n_lo] = r[:n_lo]
        ix_hi[g, :n_hi] = r[n_lo:] - SPLIT
        slots = np.concatenate([np.arange(n_lo), LO_T * 128 + np.arange(n_hi)])
        s_g.append(np.full(b - a, g))
        s_slot.append(slots)
        s_col.append(d)
    s_g = np.concatenate(s_g)
    s_slot = np.concatenate(s_slot)
    s_col = np.concatenate(s_col)
    S = np.zeros((GBLK, 128, TPB, 128), np.uint8)
    S[s_g, s_slot % 128, s_slot // 128, s_col] = 0x38  # 1.0 in fp8e4m3
    ST = np.ascontiguousarray(S.transpose(0, 3, 2, 1))
    idxb = np.ascontiguousarray(
        np.concatenate([_wrap_idx(ix_lo), _wrap_idx(ix_hi)], axis=2)
    ).view(np.uint8)
    return np.concatenate(
        [idxb, S.reshape(GBLK, 128, TPB * 128), ST.reshape(GBLK, 128, TPB * 128)],
        axis=2,
    )


def _host_prep(x, edge_index, Wl0, Wr0, att0, b0, Wl1, Wr1, att1, b1, down_W, down_b):
    # self loops for all nodes INCLUDING pad nodes: a pad node with no edges
    # has softmax den 0 -> h = 0*inf = NaN, which poisons whole blocks
    # through the one-hot aggregation matmuls (NaN*0 = NaN).
    src = np.concatenate([edge_index[0], np.arange(NPAD, dtype=np.int64)])
    dst = np.concatenate([edge_index[1], np.arange(NPAD, dtype=np.int64)])

    ed0 = _build_layer_edata(src, dst, _row0(src))
    ed1 = _build_layer_edata(src, dst, _row1(src))

    xp = np.concatenate([x, np.zeros((NPAD - N, DIN), x.dtype)])
    xT = np.ascontiguousarray(xp.T).astype(BF)          # [256, 50176]

    per_core = []
    for c in range(NCORES):
        g0, g1 = c * NBLK, (c + 1) * NBLK
        per_core.append({
            "xTo": np.ascontiguousarray(xT[:, g0 * 128:g1 * 128]),  # [256, 6272]
            "ed0": ed0[g0:g1],
            "ed1": ed1[g0:g1],
        })

    def attblk(att):
        ab = np.zeros((HID, H), np.float32)
        for h in range(H):
            ab[h * C:(h + 1) * C, h] = att[h]
        return ab.astype(BF)

    c1 = (b0 @ (Wl1 + Wr1).T).astype(np.float32)
    cd = (b1 @ down_W.T + down_b).astype(np.float32)
    shared = {
        "xT": xT,
        "wlt0": np.ascontiguousarray(Wl0.T).astype(BF),   # [256,128]
        "wrt0": np.ascontiguousarray(Wr0.T).astype(BF),
        "wlt1": np.ascontiguousarray(Wl1.T).astype(BF),   # [128,128]
        "wrt1": np.ascontiguousarray(Wr1.T).astype(BF),
        "dwt": np.ascontiguousarray(down_W.T).astype(BF),  # [128,32]
        "att0": attblk(att0), "att1": attblk(att1),
        "c1r": np.tile(c1[None, :], (128, 1)).astype(BF),
        "cdr": np.tile(cd[None, :], (128, 1)).astype(np.float32),
        "ident": np.eye(128).astype(BF),
    }
    return per_core, shared


def _build_program():
    nc = bacc.Bacc(num_swdge_queues=4)
    inp = {}
    for nm, shape, dt in [
        ("xT", [DIN, NPAD], BF16),
        ("xTo", [DIN, NPC], BF16),
        ("wlt0", [DIN, HID], BF16), ("wrt0", [DIN, HID], BF16),
        ("wlt1", [HID, HID], BF16), ("wrt1", [HID, HID], BF16),
        ("dwt", [HID, C], BF16),
        ("att0", [HID, H], BF16), ("att1", [HID, H], BF16),
        ("c1r", [128, HID], BF16), ("cdr", [128, C], F32),
        ("ident", [128, 128], BF16),
        ("ed0", [NBLK, 128, EB], U8),
        ("ed1", [NBLK, 128, EB], U8),
    ]:
        inp[nm] = nc.dram_tensor(nm, shape, dt, kind="ExternalInput")
    y = nc.dram_tensor("y", [NPC, C], F32, kind="ExternalOutput")

    with tile.TileContext(nc) as tc:
        with (
            tc.tile_pool(name="const", bufs=1) as cp,
            tc.tile_pool(name="sb", bufs=2) as sb,
            tc.tile_pool(name="sbg", bufs=6) as sbg,
            tc.tile_pool(name="se", bufs=2) as se,
            tc.tile_pool(name="sm", bufs=2) as sm,
            tc.tile_pool(name="sed", bufs=6) as sed,
            tc.tile_pool(name="sx", bufs=2) as sx,
            tc.tile_pool(name="res", bufs=1) as res,
            tc.tile_pool(name="hp", bufs=NBLK) as hp,
            tc.tile_pool(name="psA", bufs=3, space="PSUM") as psA,
            tc.tile_pool(name="psL", bufs=1, space="PSUM") as psL,
            tc.tile_pool(name="psG", bufs=2, space="PSUM") as psG,
            tc.tile_pool(name="psP", bufs=2, space="PSUM") as psP,
            tc.tile_pool(name="dram", bufs=1, space="DRAM") as dram,
        ):
            consts = {}
            for nm in ["wlt0", "wrt0", "wlt1", "wrt1", "dwt", "att0", "att1",
                       "c1r", "cdr", "ident"]:
                if nm in ("wlt0", "wrt0"):
                    t = cp.tile([128, 2, HID], inp[nm].dtype, tag=nm)
                    nc.sync.dma_start(out=t[:],
                                      in_=inp[nm][:].rearrange("(k d) h -> d k h", k=2))
                else:
                    t = cp.tile(list(inp[nm].shape), inp[nm].dtype, tag=nm)
                    nc.sync.dma_start(out=t[:], in_=inp[nm][:])
                consts[nm] = t
            ident = consts["ident"]

            xl0_full = dram.tile([NPAD, HID], BF16)
            xl1_slab = dram.tile([NPC, HID], BF16)
            xl1_full = dram.tile([NPAD, HID], BF16)

            xr0 = res.tile([128, NBLK, HID], BF16, tag="xr0")
            xr1 = res.tile([128, NBLK, HID], BF16, tag="xr1")

            # ---- Phase A: full xl0 table computed locally; own xr0 slab ----
            xTv = inp["xT"][:].rearrange("(k d) n -> d k n", k=2)
            xTov = inp["xTo"][:].rearrange("(k d) n -> d k n", k=2)
            w0l = consts["wlt0"]
            w0r = consts["wrt0"]
            for go in range(7):
                xto = sx.tile([128, 2, 7 * 128], BF16, tag="xto")
                nc.sync.dma_start(out=xto[:],
                                  in_=xTov[:, :, go * 7 * 128:(go + 1) * 7 * 128])
                for j in range(7):
                    b = go * 7 + j
                    pr = psP.tile([128, 4, 128], F32, tag="psP")
                    for k in range(2):
                        nc.tensor.matmul(out=pr[:, 0, :],
                                         lhsT=xto[:, k, j * 128:(j + 1) * 128],
                                         rhs=w0r[:, k, :], start=(k == 0), stop=(k == 1))
                    if j % 2 == 0:
                        nc.vector.tensor_copy(out=xr0[:, b, :], in_=pr[:, 0, :])
                    else:
                        nc.scalar.activation(out=xr0[:, b, :], in_=pr[:, 0, :],
                                             func=AF.Copy)

            xl0_writes = []
            for g in range(NPAD // 1024):
                xtg = sx.tile([128, 2, 1024], BF16, tag="xtg")
                nc.sync.dma_start(out=xtg[:], in_=xTv[:, :, g * 1024:(g + 1) * 1024])
                xls = sx.tile([128, 8, 128], BF16, tag="xls")
                for half in range(2):
                    pl = psA.tile([128, 4, 128], F32, tag="psA")
                    for jj in range(4):
                        j = half * 4 + jj
                        for k in range(2):
                            nc.tensor.matmul(
                                out=pl[:, jj, :],
                                lhsT=xtg[:, k, j * 128:(j + 1) * 128],
                                rhs=w0l[:, k, :], start=(k == 0), stop=(k == 1))
                    nc.scalar.activation(out=xls[:, half * 4:half * 4 + 4, :],
                                         in_=pl[:], func=AF.Copy)
                # table rows permuted (p, j) within the group: per-partition
                # contiguous 2048B runs instead of 256B rows
                w = nc.sync.dma_start(
                    out=xl0_full[g * 1024:(g + 1) * 1024, :].rearrange(
                        "(p j) f -> p j f", p=128),
                    in_=xls[:])
                xl0_writes.append(w)

            fence_sb = sb.tile([128, 4], F32, tag="fence")
            fence0_lo = nc.gpsimd.memset(fence_sb[:], 0.0)
            for w in xl0_writes[:SPLIT // 1024]:
                add_dep(fence0_lo, w, reason="xl0 lo half complete")
            fence0_hi = nc.gpsimd.memset(fence_sb[:], 0.0)
            for w in xl0_writes[SPLIT // 1024:]:
                add_dep(fence0_hi, w, reason="xl0 hi half complete")

            qctr = [0]

            def edge_pre(b, ed_t, tab_lo, fences_lo):
                """Stage edata + issue lo-half gathers for block b."""
                edt = sed.tile([128, EB], U8, tag="ed")
                led = nc.sync.dma_start(out=edt[:], in_=ed_t[b])
                XL = sbg.tile([128, TPB, 128], BF16, tag="XL")
                st = {"edt": edt, "led": led, "XL": XL}

                def gath(t0, t1, table, ixt, ixoff, fences):
                    for c0 in range(t0, t1, GCH):
                        c1 = min(c0 + GCH, t1)
                        q = qctr[0] % 4
                        qctr[0] += 1
                        g = nc.gpsimd.dma_gather(
                            out_ap=XL[:, c0:c1, :], in_ap=table,
                            idxs_ap=ixt[:, (c0 - ixoff) * 8:(c1 - ixoff) * 8],
                            num_idxs=(c1 - c0) * 128, num_idxs_reg=(c1 - c0) * 128,
                            elem_size=128, queue_num=q)
                        add_dep(g, led, reason="gather waits idx load")
                        for f in fences:
                            add_dep(g, f, reason="gather waits table")

                st["gath"] = gath
                gath(0, LO_T, tab_lo, edt[:, 0:LO_T * 16].bitcast(I16), 0,
                     fences_lo)
                return st

            def edge_main(st, b, tab_hi, xr_slab, att_t, fences_hi,
                          out_h=None, down=None):
                edt, led, XL = st["edt"], st["led"], st["XL"]
                st["gath"](LO_T, TPB, tab_hi,
                           edt[:, LO_T * 16:IXB].bitcast(I16), LO_T, fences_hi)
                Sv = edt[:, IXB:IXB + TPB * 128].bitcast(FP8).rearrange(
                    "p (t s) -> p t s", t=TPB)
                STv = edt[:, IXB + TPB * 128:EB].bitcast(FP8).rearrange(
                    "p (t s) -> p t s", t=TPB)

                # ET[f, slot] = prelu(xr_blk.T @ ST + XL.T) per tile
                ET = se.tile([128, TPB, 128], BF16, tag="ET")
                for ch in range((TPB + 3) // 4):
                    t0, t1 = ch * 4, min(ch * 4 + 4, TPB)
                    ps = psA.tile([128, 4, 128], F32, tag="psA")
                    for t in range(t0, t1):
                        nc.tensor.matmul(out=ps[:, t - t0, :], lhsT=xr_slab[:, b, :],
                                         rhs=STv[:, t, :], start=True, stop=False)
                        nc.tensor.matmul(out=ps[:, t - t0, :], lhsT=XL[:, t, :],
                                         rhs=ident[:], start=False, stop=True)
                    nc.scalar.activation(out=ET[:, t0:t1, :], in_=ps[:, 0:t1 - t0, :],
                                         func=AF.Prelu, alpha=NEG)

                # logits [slot, h] per tile; exp into M[:, :, 128:132]
                lg = psL.tile([128, TPB * 4], F32, tag="psL")
                for t in range(TPB):
                    nc.tensor.matmul(out=lg[:, t * 4:(t + 1) * 4], lhsT=ET[:, t, :],
                                     rhs=att_t[:], start=True, stop=True)
                M = sm.tile([128, TPB, 132], BF16, tag="M")
                nc.scalar.activation(
                    out=M[:, :, 128:132],
                    in_=lg[:].rearrange("p (t h) -> p t h", t=TPB), func=AF.Exp)
                nc.vector.tensor_tensor(
                    out=M[:, :, 0:128].rearrange("p t (h c) -> p t h c", h=H),
                    in0=XL[:].rearrange("p t (h c) -> p t h c", h=H),
                    in1=M[:, :, 128:132].unsqueeze(-1).to_broadcast([128, TPB, H, C]),
                    op=mybir.AluOpType.mult)

                acc = psG.tile([128, 132], F32, tag="psG")
                for t in range(TPB):
                    nc.tensor.matmul(out=acc[:], lhsT=Sv[:, t, :], rhs=M[:, t, :],
                                     start=(t == 0), stop=(t == TPB - 1))

                rec = sb.tile([128, 4], F32, tag="rec")
                nc.vector.reciprocal(out=rec[:], in_=acc[:, 128:132])
                if out_h is not None:
                    nc.vector.tensor_tensor(
                        out=out_h[:].rearrange("p (h c) -> p h c", h=H),
                        in0=acc[:, 0:128].rearrange("p (h c) -> p h c", h=H),
                        in1=rec[:].unsqueeze(-1).to_broadcast([128, H, C]),
                        op=mybir.AluOpType.mult)
                    return None
                dwt_t, cdr_t = down
                hh = sb.tile([128, 128], BF16, tag="hh")
                nc.vector.tensor_tensor(
                    out=hh[:].rearrange("p (h c) -> p h c", h=H),
                    in0=acc[:, 0:128].rearrange("p (h c) -> p h c", h=H),
                    in1=rec[:].unsqueeze(-1).to_broadcast([128, H, C]),
                    op=mybir.AluOpType.mult)
                pp = psP.tile([128, 4, 128], F32, tag="psP")
                nc.tensor.matmul(out=pp[:, 0, :], lhsT=hh[:], rhs=ident[:],
                                 start=True, stop=True)
                hhT = sb.tile([128, 128], BF16, tag="hhT")
                nc.scalar.activation(out=hhT[:], in_=pp[:, 0, :], func=AF.Copy)
                nc.tensor.matmul(out=pp[:, 1, 0:C], lhsT=hhT[:], rhs=dwt_t[:],
                                 start=True, stop=True)
                ys = sb.tile([128, C], F32, tag="ys")
                nc.vector.tensor_add(ys[:], pp[:, 1, 0:C], cdr_t[:])
                nc.sync.dma_start(out=y[b * 128:(b + 1) * 128, :], in_=ys[:])
                return None

            # ---- Phase B: layer-0 edges; fused layer-1 projections;
            #      chunked AllGather of xl1 slab ----
            def issue_ag(c, ws):
                ag = nc.gpsimd.collective_compute(
                    "AllGather", mybir.AluOpType.bypass,
                    ins=[xl1_slab[CHB_BASE[c] * 128:CHB_BASE[c + 1] * 128, :].opt()],
                    outs=[xl1_full[CHROW_BASE[c]:CHROW_BASE[c + 1], :].opt()],
                    replica_groups=[list(range(NCORES))])
                for w in ws:
                    add_dep(ag, w, reason="chunk slab ready")
                return ag

            h_tiles = []
            ags = []
            pend = []          # (chunk, [slab writes]) awaiting issue
            chunk_writes = []
            K0 = 2
            preB = {}
            tabs0 = (xl0_full[0:SPLIT, :], xl0_full[SPLIT:NPAD, :])
            for b in range(min(K0, NBLK)):
                preB[b] = edge_pre(b, inp["ed0"], tabs0[0], [fence0_lo])
            for b in range(NBLK):
                # issue a pending chunk collective two blocks late so its
                # slab writes are long done (Pool SEQ holds waits).
                if pend and b == CHB_BASE[pend[0][0] + 1] + 1:
                    c, ws = pend.pop(0)
                    ags.append(issue_ag(c, ws))

                if b + K0 < NBLK:
                    preB[b + K0] = edge_pre(b + K0, inp["ed0"], tabs0[0],
                                            [fence0_lo])
                st = preB.pop(b)
                ht = hp.tile([128, HID], BF16, tag="h0")
                edge_main(st, b, tabs0[1], xr0, consts["att0"], [fence0_hi],
                          out_h=ht)
                h_tiles.append(ht)

                # layer-1 projections for this block
                pp = psP.tile([128, 4, 128], F32, tag="psP")
                nc.tensor.matmul(out=pp[:, 0, :], lhsT=ht[:], rhs=ident[:],
                                 start=True, stop=True)
                hT = sb.tile([128, HID], BF16, tag="hT")
                nc.scalar.activation(out=hT[:], in_=pp[:, 0, :], func=AF.Copy)
                nc.tensor.matmul(out=pp[:, 1, :], lhsT=hT[:], rhs=consts["wlt1"][:],
                                 start=True, stop=True)
                nc.tensor.matmul(out=pp[:, 2, :], lhsT=hT[:], rhs=consts["wrt1"][:],
                                 start=True, stop=True)
                sl = sb.tile([128, HID], BF16, tag="sl")
                nc.vector.tensor_copy(out=sl[:], in_=pp[:, 1, :])
                w = nc.sync.dma_start(out=xl1_slab[b * 128:(b + 1) * 128, :], in_=sl[:])
                chunk_writes.append(w)
                nc.vector.tensor_add(xr1[:, b, :], pp[:, 2, :], consts["c1r"][:])

                if b + 1 in CHB_BASE[1:]:
                    pend.append((int(np.searchsorted(CHB_BASE, b + 1)) - 1,
                                 chunk_writes))
                    chunk_writes = []

            while pend:
                c, ws = pend.pop(0)
                ags.append(issue_ag(c, ws))

            fence_lo = nc.gpsimd.memset(fence_sb[:], 0.0)
            for c in LO_AGS:
                add_dep(fence_lo, ags[c], reason="lo chunks gathered")
            fence_hi = nc.gpsimd.memset(fence_sb[:], 0.0)
            for c in HI_AGS:
                add_dep(fence_hi, ags[c], reason="hi chunks gathered")

            # ---- Phase D: layer-1 edges + down proj -> y ----
            K1 = 4
            preD = {}
            tabs1 = (xl1_full[0:SPLIT, :], xl1_full[SPLIT:NPAD, :])
            for b in range(min(K1, NBLK)):
                preD[b] = edge_pre(b, inp["ed1"], tabs1[0], [fence_lo])
            for b in range(NBLK):
                if b + K1 < NBLK:
                    preD[b + K1] = edge_pre(b + K1, inp["ed1"], tabs1[0],
                                            [fence_lo])
                st = preD.pop(b)
                edge_main(st, b, tabs1[1], xr1, consts["att1"], [fence_hi],
                          down=(consts["dwt"], consts["cdr"]))

            if os.environ.get("GAT_DBG"):
                d0 = nc.dram_tensor("dbg_xl0", [NPAD, HID], BF16,
                                    kind="ExternalOutput")
                d1 = nc.dram_tensor("dbg_xl1", [NPAD, HID], BF16,
                                    kind="ExternalOutput")
                for t in range(NPAD // 1024):
                    for src_t, dst_t in ((xl0_full, d0), (xl1_full, d1)):
                        td = sx.tile([128, 8, 128], BF16, tag="xls")
                        r = nc.sync.dma_start(
                            out=td[:],
                            in_=src_t[t * 1024:(t + 1) * 1024, :].rearrange(
                                "(j p) f -> p j f", j=8))
                        for f in (fence_lo, fence_hi):
                            add_dep(r, f, reason="dbg read after tables")
                        nc.sync.dma_start(
                            out=dst_t[t * 1024:(t + 1) * 1024, :].rearrange(
                                "(j p) f -> p j f", j=8),
                            in_=td[:])

    nc.compile()
    return nc


def kernel(**inputs):
    args = {k: np.asarray(v) for k, v in inputs.items()}
    per_core, shared = _host_prep(
        args["x"].astype(np.float32), args["edge_index"].astype(np.int64),
        args["Wl0"], args["Wr0"], args["att0"], args["b0"],
        args["Wl1"], args["Wr1"], args["att1"], args["b1"],
        args["down_W"], args["down_b"])
    if "nc" not in _CACHE:
        _CACHE["nc"] = _build_program()
    nc = _CACHE["nc"]
    in_maps = [{**shared, **pc} for pc in per_core]
    res = run_bass_kernel_spmd(nc, in_maps, list(range(NCORES)))
    yv = np.concatenate([res.results[c]["y"] for c in range(NCORES)], axis=0)
    _CACHE["last_results"] = res
    return yv[:N]
